# revision 13
# baseline (speedup 1.0000x reference)
"""VQ codebook kernel for 8 Trainium2 NeuronCores (Bass/Tile).

Problem: x (8, 4096, 512) f32, codebook (4096, 512) f32.
Returns (encoding (8,4096) int32, codebook_loss, encoder_loss, nearest (8,4096,512) f32)
matching reference:
    dist2 = ||x||^2 - 2 x.c^T + ||c||^2 ; encoding = argmin ; nearest = codebook[encoding]
    losses = mean((x - nearest)^2)   (stop_gradients make both losses numerically equal)

Strategy (data parallel over the flattened token axis, 4096 tokens/core):
  score_k = x.c_k - ||c_k||^2/2  (argmax score == argmin dist2)
  - The fp32 matmul runs at full PE rate using float32r (TF32-like, RNE to 11
    mantissa bits), compensated to ~fp32-exactness with 3 accumulated terms:
        x.c ~= xr.cr (f32r) + dx.cr (f32r) + xb.dcb (bf16)
    where xr = R11(x), dx = R11(x - xr), cr = R11(c), dcb = bf16(c - cr),
    xb = bf16(x). All 12 chunk-passes accumulate into one PSUM group; the
    -csq/2 bias is fused into the PSUM->SBUF copy (DVE tensor_add reading PSUM),
    so accumulation groups stay clean (start=True resets has_written bits;
    engine-seeded PSUM + start=False is NOT reliable on hardware).
  - argmax per token: DVE max (top-8 over the 4096 scores) + max_index.
  - nearest: dma_gather (HBM->SBUF) with the hardware's wrapped index layout
    (idx list wrapped over 16 partitions, replicated across the 8 gpsimd DSP
    cores - built via a DRAM bounce + stride-0 broadcast read), then one DMA
    to the output in token order.
  - losses: per-token max score and per-partition sum(x^2) accumulate on device;
    final tiny reduction on host: sum(x^2) - 2*sum(maxscore) over all tokens.
"""
import sys

sys.path.insert(0, "/opt/trn_rl_repo")

import numpy as np
import ml_dtypes

import concourse.bass as bass
import concourse.mybir as mybir
from concourse import bacc
from concourse.tile import TileContext
from concourse.bass_utils import run_bass_kernel_spmd

F32 = mybir.dt.float32
F32R = mybir.dt.float32r
BF16 = mybir.dt.bfloat16
U16 = mybir.dt.uint16
I16 = mybir.dt.int16
AF = mybir.ActivationFunctionType

B, S, D = 8, 4096, 512
K = 4096
NCORES = 8
NT = (B * S) // NCORES          # 4096 tokens per core
NTILES = NT // 128              # 32 token tiles
NCHUNK = D // 128               # 4 contraction chunks of 128
GCH = 16                        # gather chunks
GTOK = NT // GCH                # 512 tokens per gather chunk
GTILES = NTILES // GCH          # 4 token tiles per gather chunk

_NC_CACHE = {}
_LAST_INMAPS = None


def _round11(v):
    """RNE to 11 explicit mantissa bits (the hardware float32r rounding)."""
    u = v.astype(np.float32).view(np.uint32).astype(np.uint64)
    half = np.uint64(1 << 11)
    mask = np.uint64(0xFFFFF000)
    lsb = (u >> np.uint64(12)) & np.uint64(1)
    return ((u + half - np.uint64(1) + lsb) & mask).astype(np.uint32).view(np.float32)


def _build():
    nc = bacc.Bacc(None, target_bir_lowering=False)
    xt = nc.declare_dram_parameter("xt", [2 * NCHUNK, 128, NT], F32, False)
    xtb = nc.declare_dram_parameter("xtb", [NCHUNK, 128, NT], BF16, False)
    ct = nc.declare_dram_parameter("ct", [NCHUNK, 128, K], F32, False)
    ctb = nc.declare_dram_parameter("ctb", [NCHUNK, 128, K], BF16, False)
    cb = nc.declare_dram_parameter("cb", [K, D], F32, False)
    csqn = nc.declare_dram_parameter("csqn", [1, K], F32, False)   # -csq/2
    enc_o = nc.declare_dram_parameter("enc", [128, NTILES, 8], U16, True)
    near_o = nc.declare_dram_parameter("near", [NT, D], F32, True)
    stats_o = nc.declare_dram_parameter("stats", [128, 64], F32, True)

    with TileContext(nc) as tc:
        with (
            tc.tile_pool(name="cpool", bufs=1) as cpool,
            tc.tile_pool(name="xpool", bufs=3) as xpool,
            tc.tile_pool(name="spool", bufs=3) as spool,
            tc.tile_pool(name="gpool", bufs=2) as gpool,
            tc.tile_pool(name="misc", bufs=1) as misc,
            tc.tile_pool(name="small", bufs=3) as small,
            tc.tile_pool(name="psum", bufs=4, space="PSUM") as psp,
            tc.tile_pool(name="dram", bufs=1, space="DRAM") as dpool,
        ):
            # --- persistent tiles ---
            ctile = [cpool.tile([128, K], F32R, name=f"cr{kc}") for kc in range(NCHUNK)]
            ctileb = [cpool.tile([128, K], BF16, name=f"dcb{kc}") for kc in range(NCHUNK)]
            csqb = cpool.tile([128, K], F32, name="csqb")

            def load_x(t):
                # x tile: [128 d, 8 chunks (xr0-3, dx0-3), 128 tok] + bf16 xb
                xf = xpool.tile([128, 2 * NCHUNK, 128], F32R, tag="xf")
                nc.sync.dma_start(
                    out=xf,
                    in_=xt.ap()[:, :, t * 128:(t + 1) * 128]
                    .rearrange("c k m -> k c m").bitcast(F32R),
                )
                xb = xpool.tile([128, NCHUNK, 128], BF16, tag="xb")
                nc.sync.dma_start(
                    out=xb,
                    in_=xtb.ap()[:, :, t * 128:(t + 1) * 128].rearrange("c k m -> k c m"),
                )
                return xf, xb

            x_pre = load_x(0)       # tile-0 x first: it gates the very first matmul
            for q in range(4):      # then codebook per 1024-code block, in use order
                sl = slice(q * 1024, (q + 1) * 1024)
                for kc in range(NCHUNK):
                    nc.sync.dma_start(out=ctile[kc][:, sl],
                                      in_=ct.ap()[kc, :, sl].bitcast(F32R))
                    nc.sync.dma_start(out=ctileb[kc][:, sl],
                                      in_=ctb.ap()[kc, :, sl])
                nc.sync.dma_start(out=csqb[:, sl],
                                  in_=csqn.ap()[0, sl][None].to_broadcast([128, 1024]))
            enc8 = misc.tile([128, NTILES, 8], U16, name="enc8")
            stats = misc.tile([128, 64], F32, name="stats")
            wrap_dram = dpool.tile([GCH, 16, GTOK // 16], U16, name="wrapd")

            def quarter(xf, xb, score, q):
                ps = psp.tile([128, 1024], F32, tag="ps")
                for h in range(2):
                    col = q * 1024 + h * 512
                    pslice = ps[:, h * 512:(h + 1) * 512]
                    mm = []
                    for kc in range(NCHUNK):      # xr . cr
                        mm.append((xf[:, kc, :], ctile[kc][:, col:col + 512]))
                    for kc in range(NCHUNK):      # dx . cr
                        mm.append((xf[:, NCHUNK + kc, :], ctile[kc][:, col:col + 512]))
                    for kc in range(NCHUNK):      # xb . dcb
                        mm.append((xb[:, kc, :], ctileb[kc][:, col:col + 512]))
                    for j, (lhsT, rhs) in enumerate(mm):
                        nc.tensor.matmul(
                            pslice, lhsT, rhs,
                            start=(j == 0), stop=(j == len(mm) - 1),
                        )
                # copy out of PSUM with the -csq/2 bias fused (DVE reads PSUM)
                sl = slice(q * 1024, (q + 1) * 1024)
                nc.vector.tensor_add(score[:, sl], ps, csqb[:, sl])

            for t in range(NTILES):
                xf, xb = x_pre if t == 0 else load_x(t)
                score = spool.tile([128, K], F32, tag="score")
                for q in range(4):
                    quarter(xf, xb, score, q)

                # argmax over 4096 on DVE: top-8 values then index of the max
                m8 = small.tile([128, 8], F32, tag="m8")
                nc.vector.max(out=m8, in_=score)
                nc.vector.max_index(enc8[:, t, :], m8, score)
                # loss pieces: max score and sum(x^2) for this tile
                nc.scalar.activation(stats[:, t:t + 1], m8[:, 0:1], AF.Copy)
                sqs = small.tile([128, 512], F32, tag="sqs")
                nc.scalar.activation(
                    sqs, xf[:, 0:NCHUNK, :].rearrange("k c m -> k (c m)").bitcast(F32),
                    AF.Square, accum_out=stats[:, 32 + t:33 + t],
                )

                # gather epilogue per chunk of 4 token tiles
                if t % GTILES == GTILES - 1:
                    g = t // GTILES
                    # enc8 tile-slice -> DRAM wrapped idx layout [16, 32]
                    nc.sync.dma_start(
                        out=wrap_dram[g].rearrange("q (t r) -> r q t", r=8),
                        in_=enc8[:, g * GTILES:(g + 1) * GTILES, 0],
                    )
                    widx = gpool.tile([128, GTOK // 16], I16, tag="widx")
                    nc.sync.dma_start(
                        out=widx,
                        in_=wrap_dram[g][None]
                        .to_broadcast([8, 16, GTOK // 16]).bitcast(I16),
                    )
                    gout = gpool.tile([128, GTILES, 512], F32, tag="gout")
                    nc.gpsimd.dma_gather(
                        out_ap=gout, in_ap=cb.ap(), idxs_ap=widx,
                        num_idxs=GTOK, num_idxs_reg=GTOK, elem_size=512,
                    )
                    nc.sync.dma_start(
                        out=near_o.ap().rearrange("(c j p) d -> c p j d", p=128, j=GTILES)[g],
                        in_=gout,
                    )

            nc.sync.dma_start(out=enc_o.ap(), in_=enc8)
            nc.sync.dma_start(out=stats_o.ap(), in_=stats)
    nc.finalize()
    return nc


def _get_nc():
    if "nc" not in _NC_CACHE:
        _NC_CACHE["nc"] = _build()
    return _NC_CACHE["nc"]


def kernel(x, codebook):
    x = np.ascontiguousarray(np.asarray(x, dtype=np.float32))
    codebook = np.ascontiguousarray(np.asarray(codebook, dtype=np.float32))
    xf = x.reshape(-1, D)                       # (32768, 512)

    # --- codebook-side host prep (shared by all cores) ---
    cT = np.ascontiguousarray(codebook.T)       # (512, 4096)
    cr = _round11(cT)
    dcb = (cT.astype(np.float64) - cr.astype(np.float64)).astype(ml_dtypes.bfloat16)
    ct_in = np.ascontiguousarray(cr.reshape(NCHUNK, 128, K))
    ctb_in = np.ascontiguousarray(dcb.reshape(NCHUNK, 128, K))
    csq = (codebook.astype(np.float64) ** 2).sum(axis=1)
    csqn_in = (-csq / 2.0).astype(np.float32)[None, :]

    in_maps = []
    for c in range(NCORES):
        xs = xf[c * NT:(c + 1) * NT]            # (4096, 512)
        xT = np.ascontiguousarray(xs.T)         # (512, 4096)
        xr = _round11(xT)
        dx = _round11((xT.astype(np.float64) - xr.astype(np.float64)).astype(np.float32))
        xt_in = np.concatenate(
            [xr.reshape(NCHUNK, 128, NT), dx.reshape(NCHUNK, 128, NT)], axis=0
        )
        xtb_in = xT.astype(ml_dtypes.bfloat16).reshape(NCHUNK, 128, NT)
        in_maps.append({
            "xt": np.ascontiguousarray(xt_in),
            "xtb": np.ascontiguousarray(xtb_in),
            "ct": ct_in, "ctb": ctb_in, "cb": codebook, "csqn": csqn_in,
        })

    nc = _get_nc()
    global _LAST_INMAPS
    _LAST_INMAPS = in_maps
    results = None
    last_err = None
    for _attempt in range(3):
        try:
            results = run_bass_kernel_spmd(
                nc, in_maps, core_ids=list(range(NCORES))
            ).results
            break
        except Exception as e:  # transient device/runtime failures: retry
            last_err = e
    if results is None:
        raise last_err

    enc_full = np.empty((NCORES * NT,), np.int32)
    near_full = np.empty((NCORES * NT, D), np.float32)
    loss_sum = 0.0
    for c in range(NCORES):
        r = results[c]
        enc_full[c * NT:(c + 1) * NT] = (
            r["enc"][:, :, 0].astype(np.int64).T.reshape(-1).astype(np.int32)
        )
        near_full[c * NT:(c + 1) * NT] = r["near"]
        st = r["stats"].astype(np.float64)
        loss_sum += st[:, 32:64].sum() - 2.0 * st[:, 0:32].sum()

    loss = np.float32(loss_sum / (NCORES * NT * D))
    return (
        enc_full.reshape(B, S),
        loss,
        loss,
        near_full.reshape(B, S, D),
    )


# revision 14
# speedup vs baseline: 1.0058x; 1.0058x over previous
"""VQ codebook kernel for 8 Trainium2 NeuronCores (Bass/Tile).

Problem: x (8, 4096, 512) f32, codebook (4096, 512) f32.
Returns (encoding (8,4096) int32, codebook_loss, encoder_loss, nearest (8,4096,512) f32)
matching reference:
    dist2 = ||x||^2 - 2 x.c^T + ||c||^2 ; encoding = argmin ; nearest = codebook[encoding]
    losses = mean((x - nearest)^2)   (stop_gradients make both losses numerically equal)

Strategy (data parallel over the flattened token axis, 4096 tokens/core):
  score_k = x.c_k - ||c_k||^2/2  (argmax score == argmin dist2)
  - The fp32 matmul runs at full PE rate using float32r (TF32-like, RNE to 11
    mantissa bits), compensated to ~fp32-exactness with 3 accumulated terms:
        x.c ~= xr.cr (f32r) + dx.cr (f32r) + xb.dcb (bf16)
    where xr = R11(x), dx = R11(x - xr), cr = R11(c), dcb = bf16(c - cr),
    xb = bf16(x). All 12 chunk-passes accumulate into one PSUM group; the
    -csq/2 bias is fused into the PSUM->SBUF copy (DVE tensor_add reading PSUM),
    so accumulation groups stay clean (start=True resets has_written bits;
    engine-seeded PSUM + start=False is NOT reliable on hardware).
  - argmax per token: DVE max (top-8 over the 4096 scores) + max_index.
  - nearest: dma_gather (HBM->SBUF) with the hardware's wrapped index layout
    (idx list wrapped over 16 partitions, replicated across the 8 gpsimd DSP
    cores - built via a DRAM bounce + stride-0 broadcast read), then one DMA
    to the output in token order.
  - losses: per-token max score and per-partition sum(x^2) accumulate on device;
    final tiny reduction on host: sum(x^2) - 2*sum(maxscore) over all tokens.
"""
import sys

sys.path.insert(0, "/opt/trn_rl_repo")

import numpy as np
import ml_dtypes

import concourse.bass as bass
import concourse.mybir as mybir
from concourse import bacc
from concourse.tile import TileContext
from concourse.bass_utils import run_bass_kernel_spmd

F32 = mybir.dt.float32
F32R = mybir.dt.float32r
BF16 = mybir.dt.bfloat16
U16 = mybir.dt.uint16
I16 = mybir.dt.int16
AF = mybir.ActivationFunctionType

B, S, D = 8, 4096, 512
K = 4096
NCORES = 8
NT = (B * S) // NCORES          # 4096 tokens per core
NTILES = NT // 128              # 32 token tiles
NCHUNK = D // 128               # 4 contraction chunks of 128
GCH = 16                        # gather chunks
GTOK = NT // GCH                # 512 tokens per gather chunk
GTILES = NTILES // GCH          # 4 token tiles per gather chunk

_NC_CACHE = {}
_LAST_INMAPS = None


def _round11(v):
    """RNE to 11 explicit mantissa bits (the hardware float32r rounding)."""
    u = v.astype(np.float32).view(np.uint32).astype(np.uint64)
    half = np.uint64(1 << 11)
    mask = np.uint64(0xFFFFF000)
    lsb = (u >> np.uint64(12)) & np.uint64(1)
    return ((u + half - np.uint64(1) + lsb) & mask).astype(np.uint32).view(np.float32)


def _build():
    nc = bacc.Bacc(None, target_bir_lowering=False)
    xt = nc.declare_dram_parameter("xt", [2 * NCHUNK, 128, NT], F32, False)
    xtb = nc.declare_dram_parameter("xtb", [NCHUNK, 128, NT], BF16, False)
    ct = nc.declare_dram_parameter("ct", [NCHUNK, 128, K], F32, False)
    ctb = nc.declare_dram_parameter("ctb", [NCHUNK, 128, K], BF16, False)
    cb = nc.declare_dram_parameter("cb", [K, D], F32, False)
    csqn = nc.declare_dram_parameter("csqn", [1, K], F32, False)   # -csq/2
    enc_o = nc.declare_dram_parameter("enc", [128, NTILES, 8], U16, True)
    near_o = nc.declare_dram_parameter("near", [NT, D], F32, True)
    stats_o = nc.declare_dram_parameter("stats", [128, 64], F32, True)

    with TileContext(nc) as tc:
        with (
            tc.tile_pool(name="cpool", bufs=1) as cpool,
            tc.tile_pool(name="xpool", bufs=3) as xpool,
            tc.tile_pool(name="spool", bufs=3) as spool,
            tc.tile_pool(name="gpool", bufs=2) as gpool,
            tc.tile_pool(name="misc", bufs=1) as misc,
            tc.tile_pool(name="small", bufs=3) as small,
            tc.tile_pool(name="psum", bufs=4, space="PSUM") as psp,
            tc.tile_pool(name="dram", bufs=1, space="DRAM") as dpool,
        ):
            # --- persistent tiles ---
            ctile = [cpool.tile([128, K], F32R, name=f"cr{kc}") for kc in range(NCHUNK)]
            ctileb = [cpool.tile([128, K], BF16, name=f"dcb{kc}") for kc in range(NCHUNK)]
            csqb = cpool.tile([128, K], F32, name="csqb")

            def load_x(t):
                # x tile: [128 d, 8 chunks (xr0-3, dx0-3), 128 tok] + bf16 xb
                xf = xpool.tile([128, 2 * NCHUNK, 128], F32R, tag="xf")
                nc.sync.dma_start(
                    out=xf,
                    in_=xt.ap()[:, :, t * 128:(t + 1) * 128]
                    .rearrange("c k m -> k c m").bitcast(F32R),
                )
                xb = xpool.tile([128, NCHUNK, 128], BF16, tag="xb")
                nc.sync.dma_start(
                    out=xb,
                    in_=xtb.ap()[:, :, t * 128:(t + 1) * 128].rearrange("c k m -> k c m"),
                )
                return xf, xb

            x_pre = load_x(0)       # tile-0 x first: it gates the very first matmul
            for q in range(4):      # then codebook per 1024-code block, in use order
                sl = slice(q * 1024, (q + 1) * 1024)
                for kc in range(NCHUNK):   # cr first: the f32r passes only need cr
                    nc.sync.dma_start(out=ctile[kc][:, sl],
                                      in_=ct.ap()[kc, :, sl].bitcast(F32R))
                for kc in range(NCHUNK):
                    nc.sync.dma_start(out=ctileb[kc][:, sl],
                                      in_=ctb.ap()[kc, :, sl])
                nc.sync.dma_start(out=csqb[:, sl],
                                  in_=csqn.ap()[0, sl][None].to_broadcast([128, 1024]))
            enc8 = misc.tile([128, NTILES, 8], U16, name="enc8")
            stats = misc.tile([128, 64], F32, name="stats")
            wrap_dram = dpool.tile([GCH, 16, GTOK // 16], U16, name="wrapd")

            def quarter(xf, xb, score, q):
                ps = psp.tile([128, 1024], F32, tag="ps")
                for h in range(2):
                    col = q * 1024 + h * 512
                    pslice = ps[:, h * 512:(h + 1) * 512]
                    mm = []
                    for kc in range(NCHUNK):      # xr . cr
                        mm.append((xf[:, kc, :], ctile[kc][:, col:col + 512]))
                    for kc in range(NCHUNK):      # dx . cr
                        mm.append((xf[:, NCHUNK + kc, :], ctile[kc][:, col:col + 512]))
                    for kc in range(NCHUNK):      # xb . dcb
                        mm.append((xb[:, kc, :], ctileb[kc][:, col:col + 512]))
                    for j, (lhsT, rhs) in enumerate(mm):
                        nc.tensor.matmul(
                            pslice, lhsT, rhs,
                            start=(j == 0), stop=(j == len(mm) - 1),
                        )
                # copy out of PSUM with the -csq/2 bias fused (DVE reads PSUM)
                sl = slice(q * 1024, (q + 1) * 1024)
                nc.vector.tensor_add(score[:, sl], ps, csqb[:, sl])

            for t in range(NTILES):
                xf, xb = x_pre if t == 0 else load_x(t)
                score = spool.tile([128, K], F32, tag="score")
                for q in range(4):
                    quarter(xf, xb, score, q)

                # argmax on DVE: max over quarters 0-2 overlaps PE's quarter-3
                # matmuls; only the last-quarter max + combine sit on the tail.
                m8a = small.tile([128, 8], F32, tag="m8a")
                nc.vector.max(out=m8a, in_=score[:, 0:3072])
                m8b = small.tile([128, 8], F32, tag="m8b")
                nc.vector.max(out=m8b, in_=score[:, 3072:4096])
                m8 = small.tile([128, 8], F32, tag="m8")
                nc.vector.tensor_max(m8[:, 0:1], m8a[:, 0:1], m8b[:, 0:1])
                nc.vector.tensor_copy(m8[:, 1:8], m8a[:, 1:8])
                nc.vector.max_index(enc8[:, t, :], m8, score)
                # loss pieces: max score and sum(x^2) for this tile
                nc.scalar.activation(stats[:, t:t + 1], m8[:, 0:1], AF.Copy)
                sqs = small.tile([128, 512], F32, tag="sqs")
                nc.scalar.activation(
                    sqs, xf[:, 0:NCHUNK, :].rearrange("k c m -> k (c m)").bitcast(F32),
                    AF.Square, accum_out=stats[:, 32 + t:33 + t],
                )

                # gather epilogue per chunk of 4 token tiles
                if t % GTILES == GTILES - 1:
                    g = t // GTILES
                    # enc8 tile-slice -> DRAM wrapped idx layout [16, 32]
                    nc.sync.dma_start(
                        out=wrap_dram[g].rearrange("q (t r) -> r q t", r=8),
                        in_=enc8[:, g * GTILES:(g + 1) * GTILES, 0],
                    )
                    widx = gpool.tile([128, GTOK // 16], I16, tag="widx")
                    nc.sync.dma_start(
                        out=widx,
                        in_=wrap_dram[g][None]
                        .to_broadcast([8, 16, GTOK // 16]).bitcast(I16),
                    )
                    gout = gpool.tile([128, GTILES, 512], F32, tag="gout")
                    nc.gpsimd.dma_gather(
                        out_ap=gout, in_ap=cb.ap(), idxs_ap=widx,
                        num_idxs=GTOK, num_idxs_reg=GTOK, elem_size=512,
                    )
                    nc.sync.dma_start(
                        out=near_o.ap().rearrange("(c j p) d -> c p j d", p=128, j=GTILES)[g],
                        in_=gout,
                    )

            nc.sync.dma_start(out=enc_o.ap(), in_=enc8)
            nc.sync.dma_start(out=stats_o.ap(), in_=stats)
    nc.finalize()
    return nc


def _get_nc():
    if "nc" not in _NC_CACHE:
        _NC_CACHE["nc"] = _build()
    return _NC_CACHE["nc"]


def kernel(x, codebook):
    x = np.ascontiguousarray(np.asarray(x, dtype=np.float32))
    codebook = np.ascontiguousarray(np.asarray(codebook, dtype=np.float32))
    xf = x.reshape(-1, D)                       # (32768, 512)

    # --- codebook-side host prep (shared by all cores) ---
    cT = np.ascontiguousarray(codebook.T)       # (512, 4096)
    cr = _round11(cT)
    dcb = (cT.astype(np.float64) - cr.astype(np.float64)).astype(ml_dtypes.bfloat16)
    ct_in = np.ascontiguousarray(cr.reshape(NCHUNK, 128, K))
    ctb_in = np.ascontiguousarray(dcb.reshape(NCHUNK, 128, K))
    csq = (codebook.astype(np.float64) ** 2).sum(axis=1)
    csqn_in = (-csq / 2.0).astype(np.float32)[None, :]

    in_maps = []
    for c in range(NCORES):
        xs = xf[c * NT:(c + 1) * NT]            # (4096, 512)
        xT = np.ascontiguousarray(xs.T)         # (512, 4096)
        xr = _round11(xT)
        dx = _round11((xT.astype(np.float64) - xr.astype(np.float64)).astype(np.float32))
        xt_in = np.concatenate(
            [xr.reshape(NCHUNK, 128, NT), dx.reshape(NCHUNK, 128, NT)], axis=0
        )
        xtb_in = xT.astype(ml_dtypes.bfloat16).reshape(NCHUNK, 128, NT)
        in_maps.append({
            "xt": np.ascontiguousarray(xt_in),
            "xtb": np.ascontiguousarray(xtb_in),
            "ct": ct_in, "ctb": ctb_in, "cb": codebook, "csqn": csqn_in,
        })

    nc = _get_nc()
    global _LAST_INMAPS
    _LAST_INMAPS = in_maps
    results = None
    last_err = None
    for _attempt in range(3):
        try:
            results = run_bass_kernel_spmd(
                nc, in_maps, core_ids=list(range(NCORES))
            ).results
            break
        except Exception as e:  # transient device/runtime failures: retry
            last_err = e
    if results is None:
        raise last_err

    enc_full = np.empty((NCORES * NT,), np.int32)
    near_full = np.empty((NCORES * NT, D), np.float32)
    loss_sum = 0.0
    for c in range(NCORES):
        r = results[c]
        enc_full[c * NT:(c + 1) * NT] = (
            r["enc"][:, :, 0].astype(np.int64).T.reshape(-1).astype(np.int32)
        )
        near_full[c * NT:(c + 1) * NT] = r["near"]
        st = r["stats"].astype(np.float64)
        loss_sum += st[:, 32:64].sum() - 2.0 * st[:, 0:32].sum()

    loss = np.float32(loss_sum / (NCORES * NT * D))
    return (
        enc_full.reshape(B, S),
        loss,
        loss,
        near_full.reshape(B, S, D),
    )
